# revision 25
# baseline (speedup 1.0000x reference)
"""Causal self-attention (single head) on 8 TRN2 NeuronCores.

Sharding: data-parallel over batch (4) x query-interleave (2).
Core c handles batch b = c//2 and 8 query blocks of 128 chosen so that
the two cores of a batch have equal causal work:
  j=0 -> blocks [0,3,4,7,8,11,12,15],  j=1 -> blocks [1,2,5,6,9,10,13,14]
Slot t of each core processes its query block against the first
256*(t+1) permuted keys (a superset of the causal requirement), with
the exact causal mask applied from per-core query/key position data.
All cores run one identical SPMD program; only input data differs.

V is never materialized.  With host-fused Wqk = Wq^T Wk and
Wvo = Wo Wv (exact given bq = bk = 0), the kernel computes
  ax  = softmax(x Wqk x_q^T / sqrt(D) + mask) @ x   (per-core queries)
  out = ax Wvo^T + bvo
applying the value+output projection on the small (1024-query) side
instead of projecting V for all 2048 keys on both cores of a pair,
which removes ~37% of the PE work of the V-based formulation.

Compute dtype: bf16 matmuls (f32 PSUM accumulate), f32 softmax stats,
bf16 output store (host widens to f32).
"""

from contextlib import ExitStack

import numpy as np
import ml_dtypes

B, S, D = 4, 2048, 1024
P = 128
ND = D // P  # 8 d (contraction) chunks
NE = D // P  # 8 output-feature chunks
NSK = S // P  # 16 key chunks
NQB = 8  # query blocks per core
SQH = NQB * P  # 1024 queries per core
J_BLOCKS = (
    [0, 3, 4, 7, 8, 11, 12, 15],
    [1, 2, 5, 6, 9, 10, 13, 14],
)
COVS = [256 * (t + 1) for t in range(NQB)]  # key coverage per slot
SCALE = 1.0 / np.sqrt(np.float32(D))  # 1/32
NEG_BIG = -1.0e30
CPAK = NQB + S + D  # packed f32 consts width: qpos | kpos | bvo

_NC = None


def _score_tiles(cov):
    """(offset, width) score tiles of <=512 columns covering [0, cov)."""
    tiles = [(off, 512) for off in range(0, cov - cov % 512, 512)]
    if cov % 512:
        tiles.append((cov - cov % 512, cov % 512))
    return tiles


def _emit(nc, tc, dr, out_d):
    import concourse.bass as bass  # noqa: F401
    from concourse import mybir

    BF = mybir.dt.bfloat16
    F32 = mybir.dt.float32
    AF = mybir.ActivationFunctionType
    Alu = mybir.AluOpType
    X = mybir.AxisListType.X

    with ExitStack() as ctx:
        const = ctx.enter_context(tc.tile_pool(name="const", bufs=1))
        # packed f32 consts: tiny qpos slice first (needed by the first
        # masks), bulky kpos/bvo loaded later off the critical path
        cpak = const.tile([P, CPAK], F32)
        nc.sync.dma_start(cpak[:, :NQB], dr["cpak"][:, :NQB])
        qpos = cpak[:, 0:NQB]
        kpos = cpak[:, NQB : NQB + S]
        bo = cpak[:, NQB + S : NQB + S + D]
        ident = const.tile([P, P], BF)
        nc.sync.dma_start(ident[:], dr["ident"])

        # persistent activation storage
        qt_pool = ctx.enter_context(tc.tile_pool(name="qt", bufs=NE))
        xt_pool = ctx.enter_context(tc.tile_pool(name="xt", bufs=NE))
        xn_pool = ctx.enter_context(tc.tile_pool(name="xn", bufs=NSK))
        wvo_pool = ctx.enter_context(tc.tile_pool(name="wvo", bufs=ND))
        QT, XT, XN, WVO = [], [], [], []

        # ---------------- phase A: Q projection ----------------
        # QT[e] = (Wqk^T x_q^T)[e-chunk] -> [128 e, 1024 q] bf16, computed
        # d-outer in 4 waves of 4 PSUM banks so weights/x stream smoothly
        # and no wave's first matmul waits on the previous wave's evicts.
        with ExitStack() as actx:
            wd_pool = actx.enter_context(tc.tile_pool(name="wd", bufs=ND))
            xq_pool = actx.enter_context(tc.tile_pool(name="xq", bufs=ND))
            psq = actx.enter_context(tc.tile_pool(name="psq", bufs=8, space="PSUM"))

            WD, XQ = [], []
            # loads, priority-ordered over the 3 DMA rings:
            #   scalar: wd[0..7] then wvT[0..7]
            #   gpsimd: xq halves (nt0 first) then xN[0..15]
            #   sync:   qpos, ident, xT h0 (e-asc), cpak rest, xT h1
            for i in range(ND):
                wdt = wd_pool.tile([P, D], BF)
                nc.scalar.dma_start(wdt[:], dr["wd"][i])
                WD.append(wdt)
                XQ.append(xq_pool.tile([P, SQH], BF, name="xq"))
            for i in range(ND):
                nc.gpsimd.dma_start(XQ[i][:, 0:512], dr["xQ"][i][:, 0:512])
            for i in range(ND):
                nc.gpsimd.dma_start(XQ[i][:, 512:1024], dr["xQ"][i][:, 512:1024])
            for e in range(NE):
                xt = xt_pool.tile([P, S], BF)
                nc.sync.dma_start(xt[:, 0:1024], dr["xT"][e][:, 0:1024])
                XT.append(xt)
            for i in range(ND):
                wvt = wvo_pool.tile([P, D], BF)
                nc.scalar.dma_start(wvt[:], dr["wvT"][i])
                WVO.append(wvt)
            for k in range(NSK):
                xn = xn_pool.tile([P, D], BF)
                nc.gpsimd.dma_start(xn[:], dr["xN"][k])
                XN.append(xn)
            nc.sync.dma_start(cpak[:, NQB:], dr["cpak"][:, NQB:])
            for e in range(NE):
                nc.sync.dma_start(XT[e][:, 1024:2048], dr["xT"][e][:, 1024:2048])

            for e in range(NE):
                QT.append(qt_pool.tile([P, SQH], BF, name="qt"))

            for w in range(4):  # wave = (nt, e-half)
                nt, eh = divmod(w, 2)
                es = [eh * 4 + u for u in range(4)]
                pss = {
                    e: psq.tile([P, 512], F32, tag="psq", name="psq") for e in es
                }
                rhs_nt = slice(nt * 512, (nt + 1) * 512)
                for i in range(ND):
                    for e in es:
                        nc.tensor.matmul(
                            pss[e][:],
                            WD[i][:, e * P : (e + 1) * P],
                            XQ[i][:, rhs_nt],
                            start=(i == 0),
                            stop=(i == ND - 1),
                        )
                # evicts: last wave split off scalar so phase-B softmax
                # isn't queued behind them
                for e in es:
                    dst = QT[e][:, rhs_nt]
                    if w < 3:
                        nc.scalar.activation(dst, pss[e][:], AF.Copy)
                    else:
                        nc.vector.tensor_copy(dst, pss[e][:])

        # ---------------- phase B: attention + output projection ----------------
        # 4-stage software pipeline over query blocks (small blocks first):
        #   S(t): score matmuls + mask + softmax + PE-transpose of weights
        #   A(t): attended_x = softmax @ x  (rinv-scaled evict to bf16)
        #   T(t): PE-transpose of attended_x
        #   O(t): output projection vs Wvo + bias + store
        # PE program order S(i);A(i-1);T,O(i-2) keeps the PE dense while
        # softmax/evict latencies hide under neighboring matmuls.
        with ExitStack() as bctx:
            sp = bctx.enter_context(tc.tile_pool(name="s_sb", bufs=2))
            wp = bctx.enter_context(tc.tile_pool(name="w_sb", bufs=2))
            wtp = bctx.enter_context(tc.tile_pool(name="wt_sb", bufs=2))
            axp = bctx.enter_context(tc.tile_pool(name="ax_sb", bufs=2))
            axtp = bctx.enter_context(tc.tile_pool(name="axt_sb", bufs=2))
            outp = bctx.enter_context(tc.tile_pool(name="out_sb", bufs=2))
            stat = bctx.enter_context(tc.tile_pool(name="stat", bufs=3))
            ps_s = bctx.enter_context(tc.tile_pool(name="ps_s", bufs=2, space="PSUM"))
            ps_t = bctx.enter_context(tc.tile_pool(name="ps_t", bufs=2, space="PSUM"))
            ps_a = bctx.enter_context(tc.tile_pool(name="ps_a", bufs=2, space="PSUM"))
            ps_o = bctx.enter_context(tc.tile_pool(name="ps_o", bufs=2, space="PSUM"))

            def emit_scores(t, mid=None):
                cov = COVS[t]
                s_sb = sp.tile([P, cov], F32, tag="s")
                # mask term: (k > q) * -1e30, written into s_sb
                nc.vector.tensor_scalar(
                    s_sb[:],
                    kpos[:, :cov],
                    qpos[:, t : t + 1],
                    NEG_BIG,
                    op0=Alu.is_gt,
                    op1=Alu.mult,
                )
                tiles = _score_tiles(cov)
                for idx, (off, wdt) in enumerate(tiles):
                    if mid is not None and idx == len(tiles) - 1:
                        # slot the previous block's attend matmuls ahead
                        # of the last (xT-h1) tile so its DMA gets extra
                        # runway
                        mid()
                    ps = ps_s.tile([P, wdt], F32, tag="ps_s")
                    for e in range(NE):
                        nc.tensor.matmul(
                            ps[:],
                            QT[e][:, t * P : (t + 1) * P],
                            XT[e][:, off : off + wdt],
                            start=(e == 0),
                            stop=(e == NE - 1),
                        )
                    nc.vector.tensor_tensor(
                        s_sb[:, off : off + wdt],
                        ps[:],
                        s_sb[:, off : off + wdt],
                        op=Alu.add,
                    )
                negm = stat.tile([P, 1], F32, tag="negm")
                nc.vector.tensor_reduce(
                    negm[:], s_sb[:], axis=X, op=Alu.max, negate=True
                )
                negm32 = stat.tile([P, 1], F32, tag="negm32")
                nc.vector.tensor_scalar_mul(negm32[:], negm[:], float(SCALE))
                w_sb = wp.tile([P, cov], BF, tag="w")
                lsum = stat.tile([P, 1], F32, tag="lsum")
                nc.scalar.activation(
                    w_sb[:],
                    s_sb[:],
                    AF.Exp,
                    bias=negm32[:],
                    scale=float(SCALE),
                    accum_out=lsum[:],
                )
                rinv = stat.tile([P, 1], F32, tag="rinv")
                nc.vector.reciprocal(rinv[:], lsum[:])
                # weight transposes on PE (matmul transpose mode)
                K = cov // P
                wT = wtp.tile([P, cov], BF, tag="wt")
                for k in range(K):
                    pt = ps_t.tile([P, P], BF, tag="pt", name="pt")
                    nc.tensor.transpose(pt[:], w_sb[:, k * P : (k + 1) * P], ident[:])
                    nc.vector.tensor_copy(wT[:, k * P : (k + 1) * P], pt[:])
                return {"t": t, "wT": wT, "rinv": rinv}

            def emit_attend(st):
                t = st["t"]
                cov = COVS[t]
                K = cov // P
                wT, rinv = st["wT"], st["rinv"]
                ax = axp.tile([P, D], BF, tag="ax")
                for nt in range(2):
                    pa = ps_a.tile([P, 512], F32, tag="pa", name="pa")
                    for k in range(K):
                        nc.tensor.matmul(
                            pa[:],
                            wT[:, k * P : (k + 1) * P],
                            XN[k][:, nt * 512 : (nt + 1) * 512],
                            start=(k == 0),
                            stop=(k == K - 1),
                        )
                    # ax = psum * rinv (softmax normalize), bf16 evict
                    nc.scalar.activation(
                        ax[:, nt * 512 : (nt + 1) * 512],
                        pa[:],
                        AF.Copy,
                        bias=0.0,
                        scale=rinv[:],
                    )
                st["ax"] = ax

            def emit_proj(st):
                t = st["t"]
                ax = st["ax"]
                axT = axtp.tile([P, D], BF, tag="axt")
                for jd in range(ND):
                    ptx = ps_t.tile([P, P], BF, tag="pt", name="pt")
                    nc.tensor.transpose(
                        ptx[:], ax[:, jd * P : (jd + 1) * P], ident[:]
                    )
                    dst = axT[:, jd * P : (jd + 1) * P]
                    if jd % 2:
                        nc.vector.tensor_copy(dst, ptx[:])
                    else:
                        nc.scalar.activation(dst, ptx[:], AF.Copy)
                outsb = outp.tile([P, D], BF, tag="o")
                for nt in range(2):
                    po = ps_o.tile([P, 512], F32, tag="po", name="po")
                    for jd in range(ND):
                        nc.tensor.matmul(
                            po[:],
                            axT[:, jd * P : (jd + 1) * P],
                            WVO[jd][:, nt * 512 : (nt + 1) * 512],
                            start=(jd == 0),
                            stop=(jd == ND - 1),
                        )
                    # fused evict: out = psum + bvo, bf16 store (host
                    # widens back to f32)
                    nc.vector.tensor_tensor(
                        outsb[:, nt * 512 : (nt + 1) * 512],
                        po[:],
                        bo[:, nt * 512 : (nt + 1) * 512],
                        op=Alu.add,
                    )
                    eng = nc.sync if nt == 0 else nc.gpsimd
                    eng.dma_start(
                        out_d[t][:, nt * 512 : (nt + 1) * 512],
                        outsb[:, nt * 512 : (nt + 1) * 512],
                    )

            states = []
            for i in range(NQB):  # small blocks first
                if i == 4:
                    # S(4)'s last tile is the first xT-h1 reader
                    states.append(
                        emit_scores(i, mid=lambda: emit_attend(states[3]))
                    )
                else:
                    states.append(emit_scores(i))
                    if i >= 1:
                        emit_attend(states[i - 1])
                if i >= 2:
                    emit_proj(states[i - 2])
            emit_attend(states[NQB - 1])
            emit_proj(states[NQB - 2])
            emit_proj(states[NQB - 1])


def build_nc():
    """Build + compile the SPMD Bass program (cached)."""
    global _NC
    if _NC is not None:
        return _NC
    from concourse import bacc, mybir
    import concourse.tile as tile

    BF = mybir.dt.bfloat16
    F32 = mybir.dt.float32

    nc = bacc.Bacc(
        "TRN2", target_bir_lowering=False, debug=False, enable_asserts=False
    )
    dr = {}

    def din(name, shape, dt):
        dr[name] = nc.dram_tensor(name, shape, dt, kind="ExternalInput").ap()

    din("xT", (NE, P, S), BF)
    din("xQ", (ND, P, SQH), BF)
    din("xN", (NSK, P, D), BF)
    din("wd", (ND, P, D), BF)
    din("wvT", (ND, P, D), BF)
    din("ident", (P, P), BF)
    din("cpak", (P, CPAK), F32)
    out_d = nc.dram_tensor("out_c", (NQB, P, D), BF, kind="ExternalOutput").ap()

    with tile.TileContext(nc) as tc:
        _emit(nc, tc, dr, out_d)
    nc.compile()
    _NC = nc
    return nc


def make_in_maps(x, Wq, bq, Wk, bk, Wv, bv, Wo, bo):
    """Host-side sharding: per-core input dicts (bf16 compute operands)."""
    bf16 = ml_dtypes.bfloat16
    f32 = np.float32

    # host-fused weights (f32 GEMMs, exact up to fp32):
    #   scores = (x Wq^T)(x Wk^T)^T = x (Wq^T Wk) x^T       -> Wqk
    #   out    = softmax(..) (x Wv^T) Wo^T = (softmax(..) x) (Wo Wv)^T
    # so K and V never materialize on-chip.
    # Requires bq = bk = 0 (guaranteed by the problem spec).
    Wqk = Wq.T.astype(np.float32) @ Wk.astype(np.float32)  # [d1, d2]
    Wvo = Wo.astype(np.float32) @ Wv.astype(np.float32)  # [e, d]
    bvo = Wo.astype(np.float32) @ bv.astype(np.float32) + bo.astype(np.float32)
    wd_c = np.ascontiguousarray(Wqk).reshape(ND, P, D).astype(bf16)
    wv_c = np.ascontiguousarray(Wvo.T).reshape(ND, P, D).astype(bf16)
    bo_b = np.broadcast_to(bvo, (P, D))
    ident = np.eye(P, dtype=bf16)

    in_maps = []
    for c in range(8):
        b, j = c // 2, c % 2
        blocks = J_BLOCKS[j]
        other = J_BLOCKS[1 - j]
        # key permutation: slot t holds [my block t | peer block t], so
        # the first 256(t+1) permuted keys cover every true key <= any
        # query in my block t
        perm = np.concatenate(
            [
                np.r_[P * blocks[t] : P * (blocks[t] + 1),
                      P * other[t] : P * (other[t] + 1)]
                for t in range(NQB)
            ]
        )
        qsel = np.concatenate(
            [np.r_[P * g : P * (g + 1)] for g in blocks]
        )
        xTb = np.ascontiguousarray(x[b].T[:, perm])  # [D, S] permuted keys
        xQb = np.ascontiguousarray(x[b].T[:, qsel])  # [D, 1024] my queries
        xNb = np.ascontiguousarray(x[b][perm, :])  # [S, D] permuted keys
        qpos = (np.array(blocks, dtype=f32) * P)[None, :] + np.arange(
            P, dtype=f32
        )[:, None]
        kpos = np.broadcast_to(perm.astype(f32), (P, S))
        cpak = np.concatenate([qpos, kpos, bo_b], axis=1)
        assert cpak.shape == (P, CPAK)
        in_maps.append(
            {
                "xT": xTb.reshape(ND, P, S).astype(bf16),
                "xQ": xQb.reshape(ND, P, SQH).astype(bf16),
                "xN": xNb.reshape(NSK, P, D).astype(bf16),
                "wd": wd_c,
                "wvT": wv_c,
                "cpak": np.ascontiguousarray(cpak.astype(f32)),
                "ident": ident,
            }
        )
    return in_maps


def assemble_out(results):
    out = np.empty((B, S, D), dtype=np.float32)
    for c in range(8):
        b, j = c // 2, c % 2
        blocks = J_BLOCKS[j]
        oc = np.asarray(results[c]["out_c"]).astype(np.float32)  # bf16 store
        for t, g in enumerate(blocks):
            out[b, P * g : P * (g + 1), :] = oc[t]
    return out


def kernel(x, Wq, bq, Wk, bk, Wv, bv, Wo, bo):
    from concourse.bass_utils import run_bass_kernel_spmd

    nc = build_nc()
    in_maps = make_in_maps(x, Wq, bq, Wk, bk, Wv, bv, Wo, bo)
    res = run_bass_kernel_spmd(nc, in_maps, core_ids=list(range(8)))
    return assemble_out(res.results)


# revision 26
# speedup vs baseline: 1.0179x; 1.0179x over previous
"""Causal self-attention (single head) on 8 TRN2 NeuronCores.

Sharding: data-parallel over batch (4) x query-interleave (2).
Core c handles batch b = c//2 and 8 query blocks of 128 chosen so that
the two cores of a batch have equal causal work:
  j=0 -> blocks [0,3,4,7,8,11,12,15],  j=1 -> blocks [1,2,5,6,9,10,13,14]
Slot t of each core processes its query block against the first
256*(t+1) permuted keys (a superset of the causal requirement), with
the exact causal mask applied from per-core query/key position data.
All cores run one identical SPMD program; only input data differs.

V is never materialized.  With host-fused Wqk = Wq^T Wk and
Wvo = Wo Wv (exact given bq = bk = 0), the kernel computes
  ax  = softmax(x Wqk x_q^T / sqrt(D) + mask) @ x   (per-core queries)
  out = ax Wvo^T + bvo
applying the value+output projection on the small (1024-query) side
instead of projecting V for all 2048 keys on both cores of a pair,
which removes ~37% of the PE work of the V-based formulation.

Compute dtype: bf16 matmuls (f32 PSUM accumulate), f32 softmax stats,
bf16 output store (host widens to f32).
"""

from contextlib import ExitStack

import numpy as np
import ml_dtypes

B, S, D = 4, 2048, 1024
P = 128
ND = D // P  # 8 d (contraction) chunks
NE = D // P  # 8 output-feature chunks
NSK = S // P  # 16 key chunks
NQB = 8  # query blocks per core
SQH = NQB * P  # 1024 queries per core
J_BLOCKS = (
    [0, 3, 4, 7, 8, 11, 12, 15],
    [1, 2, 5, 6, 9, 10, 13, 14],
)
COVS = [256 * (t + 1) for t in range(NQB)]  # key coverage per slot
SCALE = 1.0 / np.sqrt(np.float32(D))  # 1/32
NEG_BIG = -1.0e30
CPAK = NQB + S + D  # packed f32 consts width: qpos | kpos | bvo

_NC = None


def _score_tiles(cov):
    """(offset, width) score tiles of <=512 columns covering [0, cov)."""
    tiles = [(off, 512) for off in range(0, cov - cov % 512, 512)]
    if cov % 512:
        tiles.append((cov - cov % 512, cov % 512))
    return tiles


def _emit(nc, tc, dr, out_d):
    import concourse.bass as bass  # noqa: F401
    from concourse import mybir

    BF = mybir.dt.bfloat16
    F32 = mybir.dt.float32
    AF = mybir.ActivationFunctionType
    Alu = mybir.AluOpType
    X = mybir.AxisListType.X

    with ExitStack() as ctx:
        const = ctx.enter_context(tc.tile_pool(name="const", bufs=1))
        # packed f32 consts: tiny qpos slice first (needed by the first
        # masks), bulky kpos/bvo loaded later off the critical path
        cpak = const.tile([P, CPAK], F32)
        nc.sync.dma_start(cpak[:, :NQB], dr["cpak"][:, :NQB])
        qpos = cpak[:, 0:NQB]
        kpos = cpak[:, NQB : NQB + S]
        bo = cpak[:, NQB + S : NQB + S + D]
        ident = const.tile([P, P], BF)
        nc.sync.dma_start(ident[:], dr["ident"])

        # persistent activation storage
        qt_pool = ctx.enter_context(tc.tile_pool(name="qt", bufs=NE))
        xt_pool = ctx.enter_context(tc.tile_pool(name="xt", bufs=NE))
        xn_pool = ctx.enter_context(tc.tile_pool(name="xn", bufs=NSK))
        wvo_pool = ctx.enter_context(tc.tile_pool(name="wvo", bufs=ND))
        QT, XT, XN, WVO = [], [], [], []

        # ---------------- phase A: Q projection ----------------
        # QT[e] = (Wqk^T x_q^T)[e-chunk] -> [128 e, 1024 q] bf16, computed
        # d-outer in 4 waves of 4 PSUM banks so weights/x stream smoothly
        # and no wave's first matmul waits on the previous wave's evicts.
        with ExitStack() as actx:
            wd_pool = actx.enter_context(tc.tile_pool(name="wd", bufs=ND))
            xq_pool = actx.enter_context(tc.tile_pool(name="xq", bufs=ND))
            psq = actx.enter_context(tc.tile_pool(name="psq", bufs=8, space="PSUM"))

            WD, XQ = [], []
            # loads, priority-ordered over the 3 DMA rings:
            #   scalar: wd[0..7] then wvT[0..7]
            #   gpsimd: xq halves (nt0 first) then xN[0..15]
            #   sync:   qpos, ident, xT h0 (e-asc), cpak rest, xT h1
            for i in range(ND):
                wdt = wd_pool.tile([P, D], BF)
                nc.scalar.dma_start(wdt[:], dr["wd"][i])
                WD.append(wdt)
                XQ.append(xq_pool.tile([P, SQH], BF, name="xq"))
            for i in range(ND):
                nc.gpsimd.dma_start(XQ[i][:, 0:512], dr["xQ"][i][:, 0:512])
            for i in range(ND):
                nc.gpsimd.dma_start(XQ[i][:, 512:1024], dr["xQ"][i][:, 512:1024])
            for e in range(NE):
                xt = xt_pool.tile([P, S], BF)
                nc.sync.dma_start(xt[:, 0:1024], dr["xT"][e][:, 0:1024])
                XT.append(xt)
            for i in range(ND):
                wvt = wvo_pool.tile([P, D], BF)
                nc.scalar.dma_start(wvt[:], dr["wvT"][i])
                WVO.append(wvt)
            for k in range(NSK):
                xn = xn_pool.tile([P, D], BF)
                nc.gpsimd.dma_start(xn[:], dr["xN"][k])
                XN.append(xn)
            nc.sync.dma_start(cpak[:, NQB:], dr["cpak"][:, NQB:])
            # h1 in quarters, [1024:1536] first: S(4)/S(5)'s tiles then
            # wait on 1 MB of arrivals instead of the full 2 MB of h1
            for e in range(NE):
                nc.sync.dma_start(XT[e][:, 1024:1536], dr["xT"][e][:, 1024:1536])
            for e in range(NE):
                nc.sync.dma_start(XT[e][:, 1536:2048], dr["xT"][e][:, 1536:2048])

            for e in range(NE):
                QT.append(qt_pool.tile([P, SQH], BF, name="qt"))

            for w in range(4):  # wave = (nt, e-half)
                nt, eh = divmod(w, 2)
                es = [eh * 4 + u for u in range(4)]
                pss = {
                    e: psq.tile([P, 512], F32, tag="psq", name="psq") for e in es
                }
                rhs_nt = slice(nt * 512, (nt + 1) * 512)
                for i in range(ND):
                    for e in es:
                        nc.tensor.matmul(
                            pss[e][:],
                            WD[i][:, e * P : (e + 1) * P],
                            XQ[i][:, rhs_nt],
                            start=(i == 0),
                            stop=(i == ND - 1),
                        )
                # evicts: last wave split off scalar so phase-B softmax
                # isn't queued behind them
                for e in es:
                    dst = QT[e][:, rhs_nt]
                    if w < 3:
                        nc.scalar.activation(dst, pss[e][:], AF.Copy)
                    else:
                        nc.vector.tensor_copy(dst, pss[e][:])

        # ---------------- phase B: attention + output projection ----------------
        # 4-stage software pipeline over query blocks (small blocks first):
        #   S(t): score matmuls + mask + softmax + PE-transpose of weights
        #   A(t): attended_x = softmax @ x  (rinv-scaled evict to bf16)
        #   T(t): PE-transpose of attended_x
        #   O(t): output projection vs Wvo + bias + store
        # PE program order S(i);A(i-1);T,O(i-2) keeps the PE dense while
        # softmax/evict latencies hide under neighboring matmuls.
        with ExitStack() as bctx:
            sp = bctx.enter_context(tc.tile_pool(name="s_sb", bufs=2))
            wp = bctx.enter_context(tc.tile_pool(name="w_sb", bufs=2))
            wtp = bctx.enter_context(tc.tile_pool(name="wt_sb", bufs=2))
            axp = bctx.enter_context(tc.tile_pool(name="ax_sb", bufs=2))
            axtp = bctx.enter_context(tc.tile_pool(name="axt_sb", bufs=2))
            outp = bctx.enter_context(tc.tile_pool(name="out_sb", bufs=2))
            stat = bctx.enter_context(tc.tile_pool(name="stat", bufs=3))
            ps_s = bctx.enter_context(tc.tile_pool(name="ps_s", bufs=2, space="PSUM"))
            ps_t = bctx.enter_context(tc.tile_pool(name="ps_t", bufs=2, space="PSUM"))
            ps_a = bctx.enter_context(tc.tile_pool(name="ps_a", bufs=2, space="PSUM"))
            ps_o = bctx.enter_context(tc.tile_pool(name="ps_o", bufs=2, space="PSUM"))

            def emit_scores(t, mid=None):
                cov = COVS[t]
                s_sb = sp.tile([P, cov], F32, tag="s")
                # mask term: (k > q) * -1e30, written into s_sb
                nc.vector.tensor_scalar(
                    s_sb[:],
                    kpos[:, :cov],
                    qpos[:, t : t + 1],
                    NEG_BIG,
                    op0=Alu.is_gt,
                    op1=Alu.mult,
                )
                tiles = _score_tiles(cov)
                for idx, (off, wdt) in enumerate(tiles):
                    if mid is not None and idx == len(tiles) - 1:
                        # slot the previous block's attend matmuls ahead
                        # of the last (xT-h1) tile so its DMA gets extra
                        # runway
                        mid()
                    ps = ps_s.tile([P, wdt], F32, tag="ps_s")
                    for e in range(NE):
                        nc.tensor.matmul(
                            ps[:],
                            QT[e][:, t * P : (t + 1) * P],
                            XT[e][:, off : off + wdt],
                            start=(e == 0),
                            stop=(e == NE - 1),
                        )
                    nc.vector.tensor_tensor(
                        s_sb[:, off : off + wdt],
                        ps[:],
                        s_sb[:, off : off + wdt],
                        op=Alu.add,
                    )
                negm = stat.tile([P, 1], F32, tag="negm")
                nc.vector.tensor_reduce(
                    negm[:], s_sb[:], axis=X, op=Alu.max, negate=True
                )
                negm32 = stat.tile([P, 1], F32, tag="negm32")
                nc.vector.tensor_scalar_mul(negm32[:], negm[:], float(SCALE))
                w_sb = wp.tile([P, cov], BF, tag="w")
                lsum = stat.tile([P, 1], F32, tag="lsum")
                nc.scalar.activation(
                    w_sb[:],
                    s_sb[:],
                    AF.Exp,
                    bias=negm32[:],
                    scale=float(SCALE),
                    accum_out=lsum[:],
                )
                rinv = stat.tile([P, 1], F32, tag="rinv")
                nc.vector.reciprocal(rinv[:], lsum[:])
                # weight transposes on PE (matmul transpose mode)
                K = cov // P
                wT = wtp.tile([P, cov], BF, tag="wt")
                for k in range(K):
                    pt = ps_t.tile([P, P], BF, tag="pt", name="pt")
                    nc.tensor.transpose(pt[:], w_sb[:, k * P : (k + 1) * P], ident[:])
                    nc.vector.tensor_copy(wT[:, k * P : (k + 1) * P], pt[:])
                return {"t": t, "wT": wT, "rinv": rinv}

            def emit_attend(st):
                t = st["t"]
                cov = COVS[t]
                K = cov // P
                wT, rinv = st["wT"], st["rinv"]
                ax = axp.tile([P, D], BF, tag="ax")
                for nt in range(2):
                    pa = ps_a.tile([P, 512], F32, tag="pa", name="pa")
                    for k in range(K):
                        nc.tensor.matmul(
                            pa[:],
                            wT[:, k * P : (k + 1) * P],
                            XN[k][:, nt * 512 : (nt + 1) * 512],
                            start=(k == 0),
                            stop=(k == K - 1),
                        )
                    # ax = psum * rinv (softmax normalize), bf16 evict
                    nc.scalar.activation(
                        ax[:, nt * 512 : (nt + 1) * 512],
                        pa[:],
                        AF.Copy,
                        bias=0.0,
                        scale=rinv[:],
                    )
                st["ax"] = ax

            def emit_proj(st):
                t = st["t"]
                ax = st["ax"]
                axT = axtp.tile([P, D], BF, tag="axt")
                for jd in range(ND):
                    ptx = ps_t.tile([P, P], BF, tag="pt", name="pt")
                    nc.tensor.transpose(
                        ptx[:], ax[:, jd * P : (jd + 1) * P], ident[:]
                    )
                    dst = axT[:, jd * P : (jd + 1) * P]
                    if jd % 2:
                        nc.vector.tensor_copy(dst, ptx[:])
                    else:
                        nc.scalar.activation(dst, ptx[:], AF.Copy)
                outsb = outp.tile([P, D], BF, tag="o")
                for nt in range(2):
                    po = ps_o.tile([P, 512], F32, tag="po", name="po")
                    for jd in range(ND):
                        nc.tensor.matmul(
                            po[:],
                            axT[:, jd * P : (jd + 1) * P],
                            WVO[jd][:, nt * 512 : (nt + 1) * 512],
                            start=(jd == 0),
                            stop=(jd == ND - 1),
                        )
                    # fused evict: out = psum + bvo, bf16 store (host
                    # widens back to f32)
                    nc.vector.tensor_tensor(
                        outsb[:, nt * 512 : (nt + 1) * 512],
                        po[:],
                        bo[:, nt * 512 : (nt + 1) * 512],
                        op=Alu.add,
                    )
                    eng = nc.sync if nt == 0 else nc.gpsimd
                    eng.dma_start(
                        out_d[t][:, nt * 512 : (nt + 1) * 512],
                        outsb[:, nt * 512 : (nt + 1) * 512],
                    )

            states = []
            for i in range(NQB):  # small blocks first
                if i == 4:
                    # S(4)'s last tile is the first xT-h1 reader
                    states.append(
                        emit_scores(i, mid=lambda: emit_attend(states[3]))
                    )
                else:
                    states.append(emit_scores(i))
                    if i >= 1:
                        emit_attend(states[i - 1])
                if i >= 2:
                    emit_proj(states[i - 2])
            emit_attend(states[NQB - 1])
            emit_proj(states[NQB - 2])
            emit_proj(states[NQB - 1])


def build_nc():
    """Build + compile the SPMD Bass program (cached)."""
    global _NC
    if _NC is not None:
        return _NC
    from concourse import bacc, mybir
    import concourse.tile as tile

    BF = mybir.dt.bfloat16
    F32 = mybir.dt.float32

    nc = bacc.Bacc(
        "TRN2", target_bir_lowering=False, debug=False, enable_asserts=False
    )
    dr = {}

    def din(name, shape, dt):
        dr[name] = nc.dram_tensor(name, shape, dt, kind="ExternalInput").ap()

    din("xT", (NE, P, S), BF)
    din("xQ", (ND, P, SQH), BF)
    din("xN", (NSK, P, D), BF)
    din("wd", (ND, P, D), BF)
    din("wvT", (ND, P, D), BF)
    din("ident", (P, P), BF)
    din("cpak", (P, CPAK), F32)
    out_d = nc.dram_tensor("out_c", (NQB, P, D), BF, kind="ExternalOutput").ap()

    with tile.TileContext(nc) as tc:
        _emit(nc, tc, dr, out_d)
    nc.compile()
    _NC = nc
    return nc


def make_in_maps(x, Wq, bq, Wk, bk, Wv, bv, Wo, bo):
    """Host-side sharding: per-core input dicts (bf16 compute operands)."""
    bf16 = ml_dtypes.bfloat16
    f32 = np.float32

    # host-fused weights (f32 GEMMs, exact up to fp32):
    #   scores = (x Wq^T)(x Wk^T)^T = x (Wq^T Wk) x^T       -> Wqk
    #   out    = softmax(..) (x Wv^T) Wo^T = (softmax(..) x) (Wo Wv)^T
    # so K and V never materialize on-chip.
    # Requires bq = bk = 0 (guaranteed by the problem spec).
    Wqk = Wq.T.astype(np.float32) @ Wk.astype(np.float32)  # [d1, d2]
    Wvo = Wo.astype(np.float32) @ Wv.astype(np.float32)  # [e, d]
    bvo = Wo.astype(np.float32) @ bv.astype(np.float32) + bo.astype(np.float32)
    wd_c = np.ascontiguousarray(Wqk).reshape(ND, P, D).astype(bf16)
    wv_c = np.ascontiguousarray(Wvo.T).reshape(ND, P, D).astype(bf16)
    bo_b = np.broadcast_to(bvo, (P, D))
    ident = np.eye(P, dtype=bf16)

    in_maps = []
    for c in range(8):
        b, j = c // 2, c % 2
        blocks = J_BLOCKS[j]
        other = J_BLOCKS[1 - j]
        # key permutation: slot t holds [my block t | peer block t], so
        # the first 256(t+1) permuted keys cover every true key <= any
        # query in my block t
        perm = np.concatenate(
            [
                np.r_[P * blocks[t] : P * (blocks[t] + 1),
                      P * other[t] : P * (other[t] + 1)]
                for t in range(NQB)
            ]
        )
        qsel = np.concatenate(
            [np.r_[P * g : P * (g + 1)] for g in blocks]
        )
        xTb = np.ascontiguousarray(x[b].T[:, perm])  # [D, S] permuted keys
        xQb = np.ascontiguousarray(x[b].T[:, qsel])  # [D, 1024] my queries
        xNb = np.ascontiguousarray(x[b][perm, :])  # [S, D] permuted keys
        qpos = (np.array(blocks, dtype=f32) * P)[None, :] + np.arange(
            P, dtype=f32
        )[:, None]
        kpos = np.broadcast_to(perm.astype(f32), (P, S))
        cpak = np.concatenate([qpos, kpos, bo_b], axis=1)
        assert cpak.shape == (P, CPAK)
        in_maps.append(
            {
                "xT": xTb.reshape(ND, P, S).astype(bf16),
                "xQ": xQb.reshape(ND, P, SQH).astype(bf16),
                "xN": xNb.reshape(NSK, P, D).astype(bf16),
                "wd": wd_c,
                "wvT": wv_c,
                "cpak": np.ascontiguousarray(cpak.astype(f32)),
                "ident": ident,
            }
        )
    return in_maps


def assemble_out(results):
    out = np.empty((B, S, D), dtype=np.float32)
    for c in range(8):
        b, j = c // 2, c % 2
        blocks = J_BLOCKS[j]
        oc = np.asarray(results[c]["out_c"]).astype(np.float32)  # bf16 store
        for t, g in enumerate(blocks):
            out[b, P * g : P * (g + 1), :] = oc[t]
    return out


def kernel(x, Wq, bq, Wk, bk, Wv, bv, Wo, bo):
    from concourse.bass_utils import run_bass_kernel_spmd

    nc = build_nc()
    in_maps = make_in_maps(x, Wq, bq, Wk, bk, Wv, bv, Wo, bo)
    res = run_bass_kernel_spmd(nc, in_maps, core_ids=list(range(8)))
    return assemble_out(res.results)
